# revision 1
# baseline (speedup 1.0000x reference)
"""Depthwise causal conv1d (W=8) with 3 interleaved weight sets, on 8 TRN2 cores.

Reference computes r/o/a = depthwise_causal_conv(x, {rtg,obs,act}_{w,b}) and
interleaves out[:, t] = {r,o,a}[:, t] by t % 3.  Only the t%3-matching third of
each conv is needed, so total work is exactly one conv: for each output t,
out[b,t,h] = sum_k x[b, t-7+k, h] * w_{t%3}[h, k] + b_{t%3}[h].

Strategy (pure batch data-parallel, B=16 -> 2 per core):
  - host pre-transposes x to channels-major fp16 and PHASE-SPLITS time by t%3
    (x_phase[p][c, u] = x[c, 3u+p], left-padded 3 zeros) so every on-chip
    access is unit-stride
  - work unit = one [128 ch, 1024 u] phase-row (a (ch-group, phase, batch)
    triple; 36 per core), split across the engines by measured throughput:
      PE   24 rows: 8 accumulating fp16 diag matmuls per 512-wide psum tile
           (~213ns each); 4 consecutive matmuls share each diag, redundant
           LDWEIGHTS deduped.  Diag matrices are built on the HOST and
           DMA'd (building them on ACT/DVE costs 19-47us of engine time)
      DVE  12 rows: 4 tensor_scalar muls (single-src ops run 2x-4x even at
           the odd fp16 offsets the tap shifts force) + the 7-add tree on
           4B-aligned tmps (2x).  scalar_tensor_tensor chains measured 2x
           with aligned operands but 1x (1.28us) on x slices: dual-src DVE
           ops need 4B-aligned operands and UP=1027 makes tap offsets odd,
           so the chain loses to mul+tree
      ACT: taps 0..3 of every DVE row (activation scale-mul, emitted ahead
           of the evictions so they never wait on PE) and PE's PSUM
           evictions ([128,1024] spanning both psum banks, fused bias)
      Pool: idle on purpose - TensorTensor there shares SBUF ports with
           DVE and measurably slows DVE down (v3 regression: +38us span),
           and GPSIMD cannot read PSUM at all
  - host re-interleaves phases / transposes back / upcasts to f32.
fp16 end-to-end rel err ~8e-4 (x, w quantization + fp16 output rounding).
"""

import os
import numpy as np

B, T, H, W = 16, 3072, 768, 8
NCORES = 8
B_LOC = B // NCORES          # 2 batches per core
G = H // 128                 # 6 channel groups
U = T // 3                   # 1024 per phase
PAD = 3                      # left zero-pad per phase (covers q in {-3..0})
UP = U + PAD                 # 1027 stored per phase
NFREE = 512                  # psum tile width (one fp32 bank)
NT = U // NFREE              # 2 psum tiles per phase

_cache = {}


def _row_role(g, s, b):
    """Engine assignment for the 36 [128,1024] phase-rows: 24 PE
    (~3.6us/row) vs 12 DVE+ACT (~6.7us DVE + ~4.9us ACT/row), two vec
    rows per ch-group.  g=5's vec rows sit at s=0 so the final group
    ends with PE work and the DVE stream drains early."""
    return "vec" if s == 2 else "pe"


def _dedupe_ldweights(nc):
    """bacc lowers every 16-bit matmul to an InstLdweights + InstMatmult pair.
    The PE serializes each load (~130ns) before its matmul.  Our loop order
    makes 4 consecutive matmuls share the same diag lhsT, so drop the
    redundant reloads: remove an InstLdweights whose weights AP equals the
    previous one on the PE stream, carrying its semaphore waits onto the next
    PE instruction (reverting bacc's move_matmul_waits_to_ldweights motion).
    The 64B ISA word has one wait slot, so only dedupe when the waits fit."""
    import concourse.mybir as mybir

    removed = 0
    for fn in nc.m.functions:
        for blk in fn.blocks:
            insts = list(blk.instructions)
            drop = set()
            last_key = None
            for i, inst in enumerate(insts):
                if getattr(inst, "engine", None) != mybir.EngineType.PE:
                    continue
                tn = type(inst).__name__
                if tn == "InstLdweights":
                    a = inst.ins[0]
                    key = (a.memref, a.offset, str(a.ap), str(a.dtype))
                    si = inst.sync_info
                    my_waits = list(si.on_wait) if si is not None else []
                    has_upd = si is not None and len(si.on_update) > 0
                    if key == last_key and not has_upd:
                        nxt = None
                        for j in range(i + 1, len(insts)):
                            if getattr(insts[j], "engine", None) == mybir.EngineType.PE:
                                nxt = insts[j]
                                break
                        if nxt is not None:
                            nsi = nxt.sync_info
                            n_waits = len(nsi.on_wait) if nsi is not None else 0
                            if n_waits + len(my_waits) <= 1:
                                if my_waits:
                                    if nsi is None:
                                        nxt.sync_info = mybir.SyncInfo(
                                            on_wait=my_waits, on_update=[]
                                        )
                                    else:
                                        nsi.on_wait = list(nsi.on_wait) + my_waits
                                drop.add(i)
                                removed += 1
                                continue
                    last_key = key
                elif tn == "InstMatmult":
                    pass  # non-self-loading; PE array state unchanged
                else:
                    last_key = None  # be conservative about other PE ops
            if drop:
                blk.instructions = [x for i, x in enumerate(insts) if i not in drop]
    return removed


def _build_nc():
    import concourse.bacc as bacc
    import concourse.mybir as mybir
    import concourse.tile as tile

    nc = bacc.Bacc("TRN2", target_bir_lowering=False, debug=False)
    f32 = mybir.dt.float32
    f16 = mybir.dt.float16

    x_d = nc.dram_tensor("x", [B_LOC, G, 128, 3 * UP], f16, kind="ExternalInput").ap()
    wd_d = nc.dram_tensor("wd", [G, 128, 3, W * 128], f16, kind="ExternalInput").ap()
    w_d = nc.dram_tensor("w", [128, G * 3 * W], f32, kind="ExternalInput").ap()
    b_d = nc.dram_tensor("b", [128, G * 3], f32, kind="ExternalInput").ap()
    y_d = nc.dram_tensor("y", [B_LOC, G, 128, 3 * U], f16, kind="ExternalOutput").ap()

    # which (g, s) groups have at least one PE row (need diag weights)
    pe_groups = sorted(
        {
            (g, s)
            for g in range(G)
            for s in range(3)
            for b in range(B_LOC)
            if _row_role(g, s, b) == "pe"
        }
    )
    diags_of_g = {g: [s for s in range(3) if (g, s) in pe_groups] for g in range(G)}

    with tile.TileContext(nc) as tc:
        with (
            tc.tile_pool(name="const", bufs=1) as constp,
            tc.tile_pool(name="diag", bufs=2) as diagp,
            tc.tile_pool(name="xp", bufs=2) as xp,
            tc.tile_pool(name="op", bufs=2) as op,
            tc.tile_pool(name="dv", bufs=2) as dv,
            tc.tile_pool(name="ps", bufs=2, space="PSUM") as psp,
        ):
            wt = constp.tile([128, G * 3 * W], f32)
            bt = constp.tile([128, G * 3], f32)
            nc.sync.dma_start(wt[:], w_d[:])
            nc.sync.dma_start(bt[:], b_d[:])

            def build_diags(g):
                """DMA the host-built diagonal fp16 weight matrices for
                group g (one contiguous [128, n_s*W*128] transfer)."""
                ss = diags_of_g[g]
                if not ss:
                    return {}
                lo, hi = min(ss), max(ss)
                assert ss == list(range(lo, hi + 1))
                dt_ = diagp.tile([128, (hi - lo + 1) * W * 128], f16, tag="wd")
                nc.sync.dma_start(dt_[:], wd_d[g][:, lo : hi + 1])
                return {
                    (s, k): dt_[:, ((s - lo) * W + k) * 128 : ((s - lo) * W + k + 1) * 128]
                    for s in ss
                    for k in range(W)
                }

            def xsl(xt, s, k):
                o = s + k - (W - 1)              # tap offset in time
                p, q = o % 3, o // 3             # phase, shift within phase
                c0 = p * UP + PAD + q
                return xt[:, c0 : c0 + U]

            diags = build_diags(0)
            for g in range(G):
                next_diags = build_diags(g + 1) if g + 1 < G else None
                xts, ots = [], []
                for b in range(B_LOC):
                    xt = xp.tile([128, 3 * UP], f16, tag=f"xt{b}")
                    nc.sync.dma_start(xt[:], x_d[b, g])
                    xts.append(xt)
                    ot = op.tile([128, 3 * U], f16, tag=f"ot{b}")
                    ots.append(ot)

                def wcol(s, k):
                    c = (g * 3 + s) * W + k
                    return wt[:, c : c + 1]

                # vec-row ACT muls first: they depend only on the x DMA,
                # so ACT never makes DVE wait behind a PE eviction
                ACT_TAPS = 4
                vec_tmps = {}
                for s in range(3):
                    for b in range(B_LOC):
                        if _row_role(g, s, b) != "vec":
                            continue
                        biasv = bt[:, g * 3 + s : g * 3 + s + 1]
                        tmps = []
                        for j in range(W):
                            tv = dv.tile(
                                [128, U], f16, tag=f"dv{s}_{b}_{j}", name=f"dv{s}_{b}_{j}"
                            )
                            tmps.append(tv)
                        for k in range(ACT_TAPS):
                            nc.scalar.activation(
                                tmps[k][:], xsl(xts[b], s, k),
                                mybir.ActivationFunctionType.Identity,
                                bias=biasv if k == 0 else 0.0, scale=wcol(s, k),
                            )
                        vec_tmps[s, b] = tmps

                for s in range(3):
                    bias_ap = bt[:, g * 3 + s : g * 3 + s + 1]
                    pe_bs = [
                        b for b in range(B_LOC) if _row_role(g, s, b) == "pe"
                    ]
                    pss, ps_full = {}, {}
                    for b in pe_bs:
                        ps = psp.tile([128, NT * NFREE], f32, tag=f"ps{b}")
                        ps_full[b] = ps
                        for nt in range(NT):
                            pss[b, nt] = ps[:, nt * NFREE : (nt + 1) * NFREE]
                    # k outer: the (b, nt) matmuls of one tap share lhsT,
                    # so the deduper elides the repeated weight loads
                    for k in range(W):
                        for b in pe_bs:
                            for nt in range(NT):
                                rhs = xsl(xts[b], s, k)[:, nt * NFREE : (nt + 1) * NFREE]
                                nc.tensor.matmul(
                                    pss[b, nt], diags[s, k], rhs,
                                    start=(k == 0), stop=(k == W - 1),
                                )
                    for b in pe_bs:
                        dst = ots[b][:, s * U : (s + 1) * U]
                        nc.scalar.activation(
                            dst, ps_full[b][:], mybir.ActivationFunctionType.Identity,
                            bias=bias_ap, scale=1.0,
                        )

                # vec rows: DVE muls for taps 4..7, then the 7-add tree
                # (DVE-owned pairs first so they overlap the ACT muls)
                for (s, b), tmps in vec_tmps.items():
                    for k in range(ACT_TAPS, W):
                        nc.vector.tensor_scalar_mul(
                            tmps[k][:], xsl(xts[b], s, k), wcol(s, k)
                        )
                    dst = ots[b][:, s * U : (s + 1) * U]
                    for a_, b_ in ((4, 5), (6, 7), (4, 6), (0, 1), (2, 3), (0, 2)):
                        nc.vector.tensor_add(tmps[a_][:], tmps[a_][:], tmps[b_][:])
                    nc.vector.tensor_add(dst, tmps[0][:], tmps[4][:])
                for b in range(B_LOC):
                    nc.sync.dma_start(y_d[b, g], ots[b][:])
                if next_diags is not None:
                    diags = next_diags

    nc.compile()
    if not os.environ.get("KERNEL_NO_LDW_DEDUP"):
        n = _dedupe_ldweights(nc)
        if os.environ.get("KERNEL_VERBOSE"):
            print(f"deduped {n} ldweights")
    return nc


def _get_nc():
    if "nc" not in _cache:
        _cache["nc"] = _build_nc()
    return _cache["nc"]


def _install_ntff_hook():
    """antenv.axon_hooks is not shipped in this container; shim it so
    bass_utils can find the NTFF profile hook (trace=True path)."""
    import sys, types
    if "antenv.axon_hooks" in sys.modules:
        return
    mod = types.ModuleType("antenv.axon_hooks")
    mod._hook = None
    mod.set_axon_ntff_profile_hook = lambda h: setattr(mod, "_hook", h)
    mod.get_axon_ntff_profile_hook = lambda: mod._hook
    sys.modules["antenv.axon_hooks"] = mod
    try:
        from trn_agent_boot.trn_boot import _ntff_profile_via_ctypes
        mod._hook = _ntff_profile_via_ctypes("/opt/axon/libaxon_pjrt.so")
    except Exception:
        mod._hook = None


def kernel(x, rtg_w, rtg_b, obs_w, obs_b, act_w, act_b):
    from concourse import bass_utils

    x = np.asarray(x, dtype=np.float32)
    w_sets = [np.asarray(a, dtype=np.float32) for a in (rtg_w, obs_w, act_w)]
    b_sets = [np.asarray(a, dtype=np.float32) for a in (rtg_b, obs_b, act_b)]

    # weights laid out [128 c_local, (g*3+s)*8+k] as f32 values (per-partition
    # scalar operands for the DVE muls)
    w_all = np.zeros((128, G * 3 * W), dtype=np.float32)
    b_all = np.zeros((128, G * 3), dtype=np.float32)
    for g in range(G):
        for s in range(3):
            w_all[:, (g * 3 + s) * W : (g * 3 + s + 1) * W] = w_sets[s][g * 128 : (g + 1) * 128]
            b_all[:, g * 3 + s] = b_sets[s][g * 128 : (g + 1) * 128]
    # host-built diagonal matmul weights: wd[g, ci, s, k*128+co] is
    # w_s[g*128+ci, k] iff ci == co else 0
    wd = np.zeros((G, 128, 3, W * 128), dtype=np.float16)
    idx = np.arange(128)
    for g in range(G):
        for s in range(3):
            for k in range(W):
                wd[g, idx, s, k * 128 + idx] = w_sets[s][g * 128 + idx, k]

    in_maps = []
    for c in range(NCORES):
        xc = x[c * B_LOC : (c + 1) * B_LOC]                      # [2, T, H]
        x_t = xc.transpose(0, 2, 1).reshape(B_LOC, G, 128, U, 3)
        xph = np.zeros((B_LOC, G, 128, 3, UP), dtype=np.float16)
        xph[..., PAD:] = x_t.transpose(0, 1, 2, 4, 3)            # [b,g,c,p,u]
        in_maps.append({"x": xph.reshape(B_LOC, G, 128, 3 * UP),
                        "wd": wd, "w": w_all, "b": b_all})

    nc = _get_nc()
    trace = bool(int(os.environ.get("KERNEL_TRACE", "0")))
    if trace:
        _install_ntff_hook()
    res = bass_utils.run_bass_kernel_spmd(
        nc, in_maps, core_ids=list(range(NCORES)), trace=trace,
    )
    _cache["last_result"] = res

    out = np.empty((B, T, H), dtype=np.float32)
    for c in range(NCORES):
        y = res.results[c]["y"].astype(np.float32)               # [b,g,c,3*U]
        y = y.reshape(B_LOC, H, 3, U).transpose(0, 1, 3, 2)      # [b,H,u,p]
        y = y.reshape(B_LOC, H, T)
        out[c * B_LOC : (c + 1) * B_LOC] = y.transpose(0, 2, 1)
    return out

